# revision 43
# baseline (speedup 1.0000x reference)
"""Batchelor GPU-NUFFT forward operator on 8 Trainium2 NeuronCores.

Math (per timepoint t):
    warped  = bilinear_warp(image, flow[..., t])
    coil    = csm * warped                                  [Nc,Nx,Ny]
    out_t[c,s] = sum_{x,y} coil[c,x,y] exp(-2pi i (kx_s (x-64) + ky_s (y-64)))
    out     = sum_t out_t                                   [Nc,NS] complex64

Sharding: 8 cores = 4 timepoints x 2 sample-halves (4096 samples each).
Host unshard: sum the 4 timepoint partials per half, concat halves.

Per-core structure:
  * warp: interleaved corner table img8[x,y,8] in DRAM, 128 per-column
    [P,1] indirect DMAs on the Pool engine (the only index form the HW
    SWDGE handles), bilinear combine + coil + stationary pack afterwards.
  * NUFFT: Khatri-Rao split y = 64h + 4*yo' + yi (YI=4, two 16-yo halves).
    Phase args v[x,s] come from tiny PE matmuls ([kx;ky;1] moving against
    coefficient rows) into PSUM; the DVE does only the +-2^23 round-trick
    range reduction, with a second 0.25-shifted chain so that ONE giant
    Sin ACT per chunk-pair yields both sin and cos (no Abs, no bias
    conflicts).  All moving tiles are precomputed in bf16 during the
    gather window.  The tail then runs 36 G-matmuls/chunk back-to-back;
    scalar copies G out of PSUM so the banks recycle fast, the outer
    factor A is applied on DVE+Pool from SBUF, and the yo-reduction is an
    accumulating selector matmul.
"""

import sys

if "/opt/trn_rl_repo" not in sys.path:
    sys.path.insert(0, "/opt/trn_rl_repo")

import math

import numpy as np

import concourse.bass as bass
import concourse.tile as tile
from concourse import bacc
from concourse import mybir

P = 128
NX = 128
NCOIL = 8
NS = 8192
NT = 4
S = 4096   # samples per core (half of NS)
CH = 512   # samples per PE chunk (one PSUM bank)
CP = 1024  # samples per elementwise chunk-pair
NCHUNK = S // CH
NPAIR = S // CP
YI = 4
YO2 = 16   # yo' per half; YO = 32 total

F32 = mybir.dt.float32
F16 = mybir.dt.float16
BF16 = mybir.dt.bfloat16
F32R = mybir.dt.float32r
I32 = mybir.dt.int32
TWO_PI = float(2.0 * math.pi)
MAGIC = 12582912.0  # 1.5*2^23: (x + M) - M == round-to-nearest(x)
ALU = mybir.AluOpType
ACTF = mybir.ActivationFunctionType


def build_program(nc: bass.Bass, dbg: bool = False):
    def dbg_out(name, src_ap, shape, dtype=F32):
        if not dbg:
            return
        d = nc.dram_tensor("dbg_" + name, shape, dtype, kind="ExternalOutput").ap()
        nc.sync.dma_start(d[:], src_ap)

    image_r = nc.dram_tensor("image_r", [NX, NX], F32, kind="ExternalInput").ap()
    image_i = nc.dram_tensor("image_i", [NX, NX], F32, kind="ExternalInput").ap()
    csm_r = nc.dram_tensor("csm_r", [NCOIL, NX, NX], F32, kind="ExternalInput").ap()
    csm_i = nc.dram_tensor("csm_i", [NCOIL, NX, NX], F32, kind="ExternalInput").ap()
    kx_d = nc.dram_tensor("kx", [S], F32, kind="ExternalInput").ap()
    ky_d = nc.dram_tensor("ky", [S], F32, kind="ExternalInput").ap()
    flow0_d = nc.dram_tensor("flow0", [NX, NX], F32, kind="ExternalInput").ap()
    flow1_d = nc.dram_tensor("flow1", [NX, NX], F32, kind="ExternalInput").ap()
    out_r = nc.dram_tensor("out_r", [NCOIL, S], F32, kind="ExternalOutput").ap()
    out_i = nc.dram_tensor("out_i", [NCOIL, S], F32, kind="ExternalOutput").ap()
    img8_d = nc.dram_tensor("img8_scratch", [NX * NX, 8], F32, kind="Internal").ap()

    # ---------------- inline constants ----------------
    pvals = np.arange(P, dtype=np.float32)
    iota_pf_d = nc.inline_tensor(pvals.reshape(P, 1), name="c_iota_pf").ap()
    half_pi_d = nc.inline_tensor(np.full((P, 1), math.pi / 2, np.float32),
                                 name="c_half_pi").ap()
    jrow_d = nc.inline_tensor(np.tile(np.arange(NX, dtype=np.float32), (P, 1)),
                              name="c_jrow").ap()
    sel_np = (np.arange(P)[:, None] // YO2 == np.arange(NCOIL)[None, :]).astype(
        np.float32)
    sel_d = nc.inline_tensor(sel_np, name="c_sel").ap()

    # arg-matmul coefficient rows (stationary [5, 128]);
    # moving = [kxh; kxl; kyh; kyl; 1] (hi/lo split defeats f32r rounding)
    xm64 = (pvals - 64.0).astype(np.float32)
    wv_list = []
    for yi in range(YI):
        for shift in (0.0, -0.25):
            w = np.zeros((5, P), np.float32)
            w[0] = xm64
            w[1] = xm64
            w[2] = float(yi - 64)
            w[3] = float(yi - 64)
            w[4] = shift
            wv_list.append(w)
    wA_list = []
    for h in range(2):
        w = np.zeros((5, P), np.float32)
        coefA = 4.0 * (np.arange(P) % YO2) + 64.0 * h
        w[2] = coefA
        w[3] = coefA
        wA_list.append(w)
    wv_d = nc.inline_tensor(np.stack(wv_list, 1).reshape(5, YI * 2 * P),
                            name="c_wv").ap()      # [5, (yi, chain, p)]
    wA_d = nc.inline_tensor(np.stack(wA_list, 1).reshape(5, 2 * P),
                            name="c_wA").ap()      # [5, (h, p)]

    with tile.TileContext(nc) as tc, \
         tc.tile_pool(name="pp", bufs=1) as pp, \
         tc.tile_pool(name="big", bufs=1) as bp:

        # ---- constants ----
        iota_pf = pp.tile([P, 1], F32)
        nc.sync.dma_start(iota_pf[:], iota_pf_d[:])
        half_pi = pp.tile([P, 1], F32)
        nc.sync.dma_start(half_pi[:], half_pi_d[:])
        jrow = pp.tile([P, NX], F32)
        nc.sync.dma_start(jrow[:], jrow_d[:])
        self32 = pp.tile([P, NCOIL], F32)
        nc.sync.dma_start(self32[:], sel_d[:])
        sel = pp.tile([P, NCOIL], BF16)
        nc.vector.tensor_copy(sel[:], self32[:])
        wvf = pp.tile([5, YI * 2 * P], F32)
        nc.sync.dma_start(wvf[:], wv_d[:])
        wv = pp.tile([5, YI * 2 * P], F32R)
        nc.vector.tensor_copy(wv[:], wvf[:])
        wAf = pp.tile([5, 2 * P], F32)
        nc.sync.dma_start(wAf[:], wA_d[:])
        wA = pp.tile([5, 2 * P], F32R)
        nc.vector.tensor_copy(wA[:], wAf[:])

        def build_mvp(pool, cp):
            """Per-pair moving rows [kxh; kxl; kyh; kyl; 1] in f32r."""
            c0 = cp * CP
            stage = pool.tile([3, CP], F32, tag="mvstage", bufs=1)
            nc.vector.memset(stage[:], 1.0)
            nc.scalar.dma_start(stage[0:1, :],
                                kx_d[c0:c0 + CP].rearrange("(p s) -> p s", p=1))
            nc.scalar.dma_start(stage[1:2, :],
                                ky_d[c0:c0 + CP].rearrange("(p s) -> p s", p=1))
            hi = pool.tile([3, CP], F32R, tag="mvhi", bufs=1)
            nc.vector.tensor_copy(hi[:], stage[:])
            lo = pool.tile([2, CP], F32R, tag="mvlo", bufs=1)
            nc.vector.tensor_tensor(lo[:], stage[0:2, :], hi[0:2, :],
                                    op=ALU.subtract)
            mvp = pool.tile([5, CP], F32R, tag="mvp", bufs=2)
            nc.scalar.dma_start(mvp[0:1, :], hi[0:1, :])
            nc.scalar.dma_start(mvp[1:2, :], lo[0:1, :])
            nc.scalar.dma_start(mvp[2:3, :], hi[1:2, :])
            nc.scalar.dma_start(mvp[3:4, :], lo[1:2, :])
            nc.scalar.dma_start(mvp[4:5, :], hi[2:3, :])
            return mvp

        # persistent trig products
        kr = bp.tile([P, NPAIR, YI, 2, CP], BF16)  # [...,0]=-sin, [...,1]=cos
        m2A = bp.tile([P, NPAIR, 2, CP], F16)      # A-factor reduced args
        RA = bp.tile([P, YI, 2, 2, 128], BF16)     # kri-mult [-Im | Re]
        RB = bp.tile([P, YI, 2, 2, 128], BF16)     # krr-mult [ Re | Im]

        with tc.tile_pool(name="wp", bufs=1) as wp, \
             tc.tile_pool(name="psa", bufs=1, space="PSUM") as psa:
            # ================ warp index math + table + gather ==============
            fl0 = wp.tile([P, NX], F32)
            nc.sync.dma_start(fl0[:], flow0_d[:])
            fl1 = wp.tile([P, NX], F32)
            nc.sync.dma_start(fl1[:], flow1_d[:])
            img_r_sb = wp.tile([P, NX], F32)
            nc.sync.dma_start(img_r_sb[:], image_r[:])
            img_i_sb = wp.tile([P, NX], F32)
            nc.sync.dma_start(img_i_sb[:], image_i[:])

            cx = wp.tile([P, NX], F32)
            nc.vector.tensor_scalar(cx[:], fl0[:], iota_pf[:, 0:1], None,
                                    op0=ALU.add)
            cx2 = wp.tile([P, NX], F32)
            nc.vector.tensor_scalar(cx2[:], cx[:], 127.0, 0.0, op0=ALU.min,
                                    op1=ALU.max)
            cyt = wp.tile([P, NX], F32)
            nc.vector.tensor_tensor(cyt[:], fl1[:], jrow[:], op=ALU.add)
            cy2 = wp.tile([P, NX], F32)
            nc.vector.tensor_scalar(cy2[:], cyt[:], 127.0, 0.0, op0=ALU.min,
                                    op1=ALU.max)

            c5x = wp.tile([P, NX], F32)
            nc.vector.tensor_scalar(c5x[:], cx2[:], 0.5, None, op0=ALU.subtract)
            x0 = wp.tile([P, NX], F32)
            nc.vector.tensor_scalar(x0[:], c5x[:], MAGIC, MAGIC,
                                    op0=ALU.add, op1=ALU.subtract)
            wx = wp.tile([P, NX], F32)
            nc.vector.tensor_tensor(wx[:], cx2[:], x0[:], op=ALU.subtract)
            c5y = wp.tile([P, NX], F32)
            nc.vector.tensor_scalar(c5y[:], cy2[:], 0.5, None, op0=ALU.subtract)
            y0 = wp.tile([P, NX], F32)
            nc.vector.tensor_scalar(y0[:], c5y[:], MAGIC, MAGIC,
                                    op0=ALU.add, op1=ALU.subtract)
            wy = wp.tile([P, NX], F32)
            nc.vector.tensor_tensor(wy[:], cy2[:], y0[:], op=ALU.subtract)

            idxf = wp.tile([P, NX], F32)
            nc.vector.tensor_scalar(idxf[:], x0[:], 128.0, None, op0=ALU.mult)
            idxf2 = wp.tile([P, NX], F32)
            nc.vector.tensor_tensor(idxf2[:], idxf[:], y0[:], op=ALU.add)
            idx_i = wp.tile([P, NX], I32)
            nc.vector.tensor_copy(idx_i[:], idxf2[:])

            csm_r_sb = wp.tile([P, NCOIL, NX], F32)
            nc.sync.dma_start(csm_r_sb[:], csm_r.rearrange("c x y -> x c y"))
            csm_i_sb = wp.tile([P, NCOIL, NX], F32)
            nc.sync.dma_start(csm_i_sb[:], csm_i.rearrange("c x y -> x c y"))

            imgBr = wp.tile([P, NX], F32)
            nc.sync.dma_start(imgBr[0:127, :], img_r_sb[1:128, :])
            nc.sync.dma_start(imgBr[127:128, :], img_r_sb[127:128, :])
            imgBi = wp.tile([P, NX], F32)
            nc.sync.dma_start(imgBi[0:127, :], img_i_sb[1:128, :])
            nc.sync.dma_start(imgBi[127:128, :], img_i_sb[127:128, :])

            img8 = wp.tile([P, NX, 8], F32)
            for k, src in ((0, img_r_sb), (2, imgBr), (4, img_i_sb), (6, imgBi)):
                nc.vector.tensor_copy(img8[:, :, k], src[:])
                nc.vector.tensor_copy(img8[:, 0:127, k + 1], src[:, 1:128])
                nc.vector.tensor_copy(img8[:, 127:128, k + 1], src[:, 127:128])
            nc.sync.dma_start(
                img8_d.rearrange("(x y) k -> x (y k)", x=NX), img8[:])

            g8 = wp.tile([P, NX, 8], F32)
            for j in range(NX):
                call = nc.gpsimd.indirect_dma_start(
                    out=g8[:, j, :],
                    out_offset=None,
                    in_=img8_d[:],
                    in_offset=bass.IndirectOffsetOnAxis(ap=idx_i[:, j:j + 1],
                                                        axis=0),
                )
                if j % 2:
                    call.ins.queue = "qPoolDynamic1"

            # ================ trig pipeline (overlaps the gather) ===========
            for cp in range(NPAIR):
                mvc = build_mvp(wp, cp)[:]

                for yp in range(YI // 2):
                    m2p = wp.tile([P, 2, 2, CP], F16, tag="m2p", bufs=2)
                    for yi2 in range(2):
                        yi = yp * 2 + yi2
                        for ci in range(2):
                            vps = psa.tile([P, 2, CH], F32, tag="vps", bufs=4)
                            wsl = wv[:, (yi * 2 + ci) * P:(yi * 2 + ci + 1) * P]
                            nc.tensor.matmul(vps[:, 0], wsl,
                                             mvc[:, 0:CH],
                                             start=True, stop=True)
                            nc.tensor.matmul(vps[:, 1], wsl,
                                             mvc[:, CH:CP],
                                             start=True, stop=True)
                            r = wp.tile([P, CP], F32, tag="rk", bufs=1)
                            nc.vector.tensor_scalar(r[:], vps[:], MAGIC, MAGIC,
                                                    op0=ALU.add,
                                                    op1=ALU.subtract)
                            nc.vector.tensor_tensor(m2p[:, yi2, ci], vps[:],
                                                    r[:], op=ALU.subtract)
                    # wide Sin ACT per yi-pair: [...,0]=-sin(theta),
                    # [...,1]=cos(theta)
                    nc.scalar.activation(kr[:, cp, yp * 2:yp * 2 + 2], m2p[:],
                                         ACTF.Sin, scale=-TWO_PI)

                for h in range(2):
                    vps = psa.tile([P, 2, CH], F32, tag="vps", bufs=4)
                    wsl = wA[:, h * P:(h + 1) * P]
                    nc.tensor.matmul(vps[:, 0], wsl,
                                     mvc[:, 0:CH],
                                     start=True, stop=True)
                    nc.tensor.matmul(vps[:, 1], wsl,
                                     mvc[:, CH:CP],
                                     start=True, stop=True)
                    rA = wp.tile([P, CP], F32, tag="rk", bufs=1)
                    nc.vector.tensor_scalar(rA[:], vps[:], MAGIC, MAGIC,
                                            op0=ALU.add, op1=ALU.subtract)
                    nc.vector.tensor_tensor(m2A[:, cp, h], vps[:], rA[:],
                                            op=ALU.subtract)

            # ================ combine + coil + pack (after gather) ==========
            onemwx = wp.tile([P, NX], F32)
            nc.vector.tensor_scalar(onemwx[:], wx[:], -1.0, 1.0, op0=ALU.mult,
                                    op1=ALU.add)
            onemwy = wp.tile([P, NX], F32)
            nc.vector.tensor_scalar(onemwy[:], wy[:], -1.0, 1.0, op0=ALU.mult,
                                    op1=ALU.add)
            w4 = wp.tile([P, NX, 4], F32)
            nc.vector.tensor_tensor(w4[:, :, 0], onemwx[:], onemwy[:],
                                    op=ALU.mult)
            nc.vector.tensor_tensor(w4[:, :, 1], onemwx[:], wy[:], op=ALU.mult)
            nc.vector.tensor_tensor(w4[:, :, 2], wx[:], onemwy[:], op=ALU.mult)
            nc.vector.tensor_tensor(w4[:, :, 3], wx[:], wy[:], op=ALU.mult)

            warped_r = wp.tile([P, NX], F32)
            warped_i = wp.tile([P, NX], F32)
            BL = 32
            for b0 in range(0, NX, BL):
                t8r = wp.tile([P, BL, 4], F32, tag="t8r", bufs=2)
                nc.vector.tensor_tensor(t8r[:], g8[:, b0:b0 + BL, 0:4],
                                        w4[:, b0:b0 + BL], op=ALU.mult)
                nc.vector.reduce_sum(warped_r[:, b0:b0 + BL],
                                     t8r[:], axis=mybir.AxisListType.X)
                t8i = wp.tile([P, BL, 4], F32, tag="t8i", bufs=2)
                nc.vector.tensor_tensor(t8i[:], g8[:, b0:b0 + BL, 4:8],
                                        w4[:, b0:b0 + BL], op=ALU.mult)
                nc.vector.reduce_sum(warped_i[:, b0:b0 + BL],
                                     t8i[:], axis=mybir.AxisListType.X)
            dbg_out("warped_r", warped_r[:], [P, NX])
            dbg_out("warped_i", warped_i[:], [P, NX])

            wr_b = warped_r[:].rearrange("p (c y) -> p c y", c=1).to_broadcast(
                [P, NCOIL, NX])
            wi_b = warped_i[:].rearrange("p (c y) -> p c y", c=1).to_broadcast(
                [P, NCOIL, NX])

            tt1 = wp.tile([P, NCOIL, NX], F32)
            nc.vector.tensor_tensor(tt1[:], csm_r_sb[:], wr_b, op=ALU.mult)
            tt2 = wp.tile([P, NCOIL, NX], F32)
            nc.gpsimd.tensor_tensor(tt2[:], csm_i_sb[:], wi_b, op=ALU.mult)
            coilr = wp.tile([P, NCOIL, NX], F32)
            nc.vector.tensor_tensor(coilr[:], tt1[:], tt2[:], op=ALU.subtract)
            tt3 = wp.tile([P, NCOIL, NX], F32, tag="tt1")
            nc.gpsimd.tensor_tensor(tt3[:], csm_r_sb[:], wi_b, op=ALU.mult)
            tt4 = wp.tile([P, NCOIL, NX], F32, tag="tt2")
            nc.vector.tensor_tensor(tt4[:], csm_i_sb[:], wr_b, op=ALU.mult)
            coili = wp.tile([P, NCOIL, NX], F32)
            nc.vector.tensor_tensor(coili[:], tt3[:], tt4[:], op=ALU.add)
            dbg_out("coilr", coilr[:], [P, NCOIL, NX])
            dbg_out("coili", coili[:], [P, NCOIL, NX])

            # gr = (-coili)*kri + coilr*krr ; gi = coilr*kri + coili*krr
            def coil_view(t, h):
                return t[:].rearrange("p c (h yo1 yi) -> p yi h c yo1",
                                      h=2, yo1=YO2, yi=YI)[:, :, h]

            def pack_view(t, h, ri):
                return t[:, :, h, ri].rearrange("p yi (c yo1) -> p yi c yo1",
                                                c=NCOIL)

            for h in range(2):
                eng2 = nc.vector if h == 0 else nc.gpsimd
                nc.vector.tensor_scalar(pack_view(RA, h, 0), coil_view(coili, h),
                                        -1.0, None, op0=ALU.mult)
                eng2.tensor_copy(pack_view(RA, h, 1), coil_view(coilr, h))
                nc.vector.tensor_copy(pack_view(RB, h, 0), coil_view(coilr, h))
                eng2.tensor_copy(pack_view(RB, h, 1), coil_view(coili, h))

        # ================ PE tail: chunk drain ================
        with tc.tile_pool(name="tp", bufs=1) as tp, \
             tc.tile_pool(name="ps", bufs=1, space="PSUM") as ps, \
             tc.tile_pool(name="pso", bufs=1, space="PSUM") as pso:
            for ch in range(NCHUNK):
                cp, half = divmod(ch, 2)
                sl = slice(half * CH, (half + 1) * CH)
                c0 = ch * CH

                mabsA = tp.tile([P, 2, CH], F32, tag="mabsA", bufs=1)
                nc.scalar.activation(mabsA[:], m2A[:, cp, :, sl], ACTF.Abs)
                aic = tp.tile([P, 2, CH], F32, tag="aic", bufs=2)
                nc.scalar.activation(aic[:], m2A[:, cp, :, sl], ACTF.Sin,
                                     scale=-TWO_PI)
                arc = tp.tile([P, 2, CH], F32, tag="arc", bufs=2)
                nc.scalar.activation(arc[:], mabsA[:], ACTF.Sin,
                                     scale=-TWO_PI, bias=half_pi[:, 0:1])

                gr = ps.tile([P, 2, CH], F32, tag="gr", bufs=2)
                gi = ps.tile([P, 2, CH], F32, tag="gi")
                for h in range(2):
                    for yi in range(YI):
                        nc.tensor.matmul(gr[:, h], RA[:, yi, h, 0],
                                         kr[:, cp, yi, 0, sl],
                                         start=(yi == 0), stop=False)
                        nc.tensor.matmul(gr[:, h], RB[:, yi, h, 0],
                                         kr[:, cp, yi, 1, sl],
                                         start=False, stop=(yi == YI - 1))
                    for yi in range(YI):
                        nc.tensor.matmul(gi[:, h], RA[:, yi, h, 1],
                                         kr[:, cp, yi, 0, sl],
                                         start=(yi == 0), stop=False)
                        nc.tensor.matmul(gi[:, h], RB[:, yi, h, 1],
                                         kr[:, cp, yi, 1, sl],
                                         start=False, stop=(yi == YI - 1))

                # scalar copies G out of PSUM so the banks recycle fast
                grs = tp.tile([P, 2, CH], F32, tag="grs", bufs=2)
                nc.scalar.copy(grs[:], gr[:])
                gis = tp.tile([P, 2, CH], F32, tag="gis", bufs=2)
                nc.scalar.copy(gis[:], gi[:])
                if ch == 0:
                    dbg_out("gr0", grs[:], [P, 2, CH])

                pr = tp.tile([P, 2, CH], BF16, tag="pr", bufs=2)
                pi_ = tp.tile([P, 2, CH], BF16, tag="pi", bufs=2)
                t1 = tp.tile([P, 2, CH], F32, tag="t1", bufs=1)
                nc.vector.tensor_tensor(t1[:], grs[:], arc[:], op=ALU.mult)
                t2 = tp.tile([P, 2, CH], F32, tag="t2", bufs=1)
                nc.vector.tensor_tensor(t2[:], gis[:], aic[:], op=ALU.mult)
                nc.vector.tensor_tensor(pr[:], t1[:], t2[:], op=ALU.subtract)
                t3 = tp.tile([P, 2, CH], F32, tag="t3", bufs=1)
                nc.vector.tensor_tensor(t3[:], gis[:], arc[:], op=ALU.mult)
                t4 = tp.tile([P, 2, CH], F32, tag="t4", bufs=1)
                nc.vector.tensor_tensor(t4[:], grs[:], aic[:], op=ALU.mult)
                nc.vector.tensor_tensor(pi_[:], t3[:], t4[:], op=ALU.add)

                orps = pso.tile([NCOIL, 2, CH], F32, tag="osel")
                nc.tensor.matmul(orps[:, 0], sel[:], pr[:, 0], start=True,
                                 stop=False)
                nc.tensor.matmul(orps[:, 0], sel[:], pr[:, 1], start=False,
                                 stop=True)
                nc.tensor.matmul(orps[:, 1], sel[:], pi_[:, 0], start=True,
                                 stop=False)
                nc.tensor.matmul(orps[:, 1], sel[:], pi_[:, 1], start=False,
                                 stop=True)

                osr = tp.tile([NCOIL, 2, CH], F32, tag="osr", bufs=2)
                nc.scalar.copy(osr[:], orps[:])
                nc.sync.dma_start(out_r[:, c0:c0 + CH], osr[:, 0])
                nc.sync.dma_start(out_i[:, c0:c0 + CH], osr[:, 1])


_COMPILED = {}


def _get_nc():
    if "nc" not in _COMPILED:
        nc = bacc.Bacc("TRN2", debug=False, num_swdge_queues=2,
                       dynamic_dma_scratch_size=32768)
        build_program(nc)
        nc.compile()
        _COMPILED["nc"] = nc
    return _COMPILED["nc"]


def make_in_maps(image_r, image_i, csm_r, csm_i, traj, dcf, flow):
    del dcf  # unused by the operator
    in_maps = []
    for core in range(8):
        t, h = divmod(core, 2)
        sl = slice(h * S, (h + 1) * S)
        in_maps.append({
            "image_r": np.ascontiguousarray(image_r, np.float32),
            "image_i": np.ascontiguousarray(image_i, np.float32),
            "csm_r": np.ascontiguousarray(csm_r, np.float32),
            "csm_i": np.ascontiguousarray(csm_i, np.float32),
            "kx": np.ascontiguousarray(traj[sl, 0, t], np.float32),
            "ky": np.ascontiguousarray(traj[sl, 1, t], np.float32),
            "flow0": np.ascontiguousarray(flow[:, :, 0, t], np.float32),
            "flow1": np.ascontiguousarray(flow[:, :, 1, t], np.float32),
        })
    return in_maps


def combine_outputs(results):
    out = np.zeros((NCOIL, NS), np.complex64)
    for core, res in enumerate(results):
        t, h = divmod(core, 2)
        sl = slice(h * S, (h + 1) * S)
        out[:, sl] += res["out_r"].astype(np.complex64) + 1j * res["out_i"].astype(
            np.complex64)
    return out


def kernel(**inputs) -> np.ndarray:
    from concourse.bass_utils import run_bass_kernel_spmd

    nc = _get_nc()
    in_maps = make_in_maps(**inputs)
    res = run_bass_kernel_spmd(nc, in_maps, core_ids=list(range(8)))
    return combine_outputs(res.results)
